# revision 2
# baseline (speedup 1.0000x reference)
"""Multi-head causal attention (B=2, S=2048, E=1024, H=16, D=64) on 8 TRN2
NeuronCores.

Sharding (data + tensor parallel, Megatron-style):
  core c -> batch b = c // 4, head group g = c % 4 (4 heads, e' = 256 cols).
  Wq/Wk/Wv column-sharded ([256, 1024] slices), Wo row-sharded
  ([1024, 256] slice); each core produces a partial output [2048, 1024]
  (fp16) which the host sums per batch group in fp32 (the Megatron
  all-reduce) and adds bo.

Per-core device kernel (matmul operands fp16, accumulate fp32 in PSUM):
  Inputs stream in 512-column chunks ordered by consumption (K, V, Q) so
  the projections start ~4us in and hide the full 12MB input DMA.
  K^T = Wk_l x_k^T + bk  [256, 2048], Q^T likewise (tt descending so
  attention qt=3 starts first), V' = [x_v Wv_l^T + bv | 1] with the ones
  column memset (softmax denominator via PE).
  Attention in S^T orientation per (q-tile 512, head-pair chunk): S^T
  tiles [128 k, 512 q] with 2 heads in PE row groups 0-63/64-127, exp on
  ACT (1/8 scale folded), multiplicative causal mask on diagonal tiles,
  acc += V'^T @ P^T in PSUM [65, 512] whose row 64 is the denominator.
  Normalize without DMA: DVE reciprocal on the PSUM denominator row,
  GpSimd partition-broadcast, fused PSUM-read multiply into valsT (fp16).
  Each qt's O-projection is deferred and interleaved into the NEXT qt's
  attention k-loop (the attention inner loop is ACT-bound, so the PE has
  slack to absorb the O-proj matmuls); outputs DMA per 128-row tile.
"""
import sys
import os

sys.path.insert(0, "/opt/trn_rl_repo")

import numpy as np
from contextlib import ExitStack

import concourse.bass as bass  # noqa: E402
import concourse.mybir as mybir  # noqa: E402
import concourse.tile as tile  # noqa: E402
from concourse import bacc, bass_utils  # noqa: E402

bass_utils.upload_artifacts = lambda d: f"local:{d}"

B, S, E, H, D = 2, 2048, 1024, 16, 64
NCORES = 8
EL = 256  # e' columns per core (4 heads)
F32 = mybir.dt.float32
F16 = mybir.dt.float16
AF = mybir.ActivationFunctionType
NP16 = np.float16

_CACHE = {}


def _build():
    nc = bacc.Bacc("TRN2", target_bir_lowering=False, debug=False)

    xq_d = nc.dram_tensor("xqT", [E, S], F16, kind="ExternalInput")
    xk_d = nc.dram_tensor("xkT", [E, S], F16, kind="ExternalInput")
    xv_d = nc.dram_tensor("xvT", [E, S], F16, kind="ExternalInput")
    wq_d = nc.dram_tensor("wqT", [E, EL], F16, kind="ExternalInput")
    wk_d = nc.dram_tensor("wkT", [E, EL], F16, kind="ExternalInput")
    wv_d = nc.dram_tensor("wvT", [E, EL], F16, kind="ExternalInput")
    wo_d = nc.dram_tensor("woT", [EL, E], F16, kind="ExternalInput")
    bq_d = nc.dram_tensor("bq", [EL], F32, kind="ExternalInput")
    bk_d = nc.dram_tensor("bk", [EL], F32, kind="ExternalInput")
    bv_d = nc.dram_tensor("bv", [EL], F32, kind="ExternalInput")
    mask_d = nc.dram_tensor("masks", [4, 128, 512], F16, kind="ExternalInput")
    out_d = nc.dram_tensor("out", [S, E], F16, kind="ExternalOutput")

    with tile.TileContext(nc) as tc, ExitStack() as ctx:
        cpool = ctx.enter_context(tc.tile_pool(name="const", bufs=1))
        psp = ctx.enter_context(tc.tile_pool(name="psp", bufs=2, space="PSUM"))
        expp = ctx.enter_context(tc.tile_pool(name="expp", bufs=6))
        opool = ctx.enter_context(tc.tile_pool(name="op", bufs=4))
        smp = ctx.enter_context(tc.tile_pool(name="smp", bufs=4))

        xk = cpool.tile([128, 8, S], F16, tag="xk")
        xq = cpool.tile([128, 8, S], F16, tag="xq")
        xv = cpool.tile([128, 8, S], F16, tag="xv")
        KT = cpool.tile([128, 2, S], F16, tag="KT")
        QT = cpool.tile([128, 2, S], F16, tag="QT")
        VP = cpool.tile([128, 16, 4 * 66], F16, tag="VP")  # 66: 4B-aligned
        valsT = cpool.tile([128, 2, S], F16, tag="valsT")

        def load_w(dram, tag):
            w = cpool.tile([128, 8, EL], F16, tag=tag)
            nc.sync.dma_start(w[:], dram.ap().rearrange("(k p) m -> p k m", p=128))
            return w

        def load_bias_cols(dram, tag):
            b = cpool.tile([128, 2], F32, tag=tag)
            nc.sync.dma_start(b[:], dram.ap().rearrange("(c p) -> p c", p=128))
            return b

        def load_x_chunk(x_t, dram, tt):
            nc.sync.dma_start(
                x_t[:, :, tt * 512:(tt + 1) * 512],
                dram.ap().rearrange("(k p) m -> p k m", p=128)[
                    :, :, tt * 512:(tt + 1) * 512],
            )

        # ---- K projection (streamed): K^T[e', t] = Wk x_k^T + bk ----
        wk = load_w(wk_d, "wk")
        bkt = load_bias_cols(bk_d, "bkt")
        for tt in range(4):
            load_x_chunk(xk, xk_d, tt)
        for tt in range(4):
            for c in range(2):
                ps = psp.tile([128, 512], F32, tag="lg", bufs=3,
                              name=f"kps{c}_{tt}")
                for k in range(8):
                    nc.tensor.matmul(
                        ps[:],
                        lhsT=wk[:, k, c * 128:(c + 1) * 128],
                        rhs=xk[:, k, tt * 512:(tt + 1) * 512],
                        start=(k == 0), stop=(k == 7))
                nc.vector.tensor_scalar_add(
                    KT[:, c, tt * 512:(tt + 1) * 512],
                    ps[:], bkt[:, c:c + 1])

        # ---- V projection (natural layout) ----
        wv = load_w(wv_d, "wv")
        bvr = cpool.tile([1, EL], F32, tag="bvr")
        nc.sync.dma_start(bvr[:], bv_d.ap().rearrange("(p m) -> p m", p=1))
        bvb = cpool.tile([128, EL], F32, tag="bvb")
        nc.gpsimd.partition_broadcast(bvb[:], bvr[:])
        # ones columns of V' (col 64 of each 66-block)
        nc.vector.memset(
            VP[:].rearrange("p k (h x) -> p k h x", h=4)[:, :, :, 64:65], 1.0)
        for tt in range(4):
            load_x_chunk(xv, xv_d, tt)
        for t3 in range(16):
            ps = psp.tile([128, EL], F32, tag="lg", bufs=3, name=f"vps{t3}")
            for k in range(8):
                nc.tensor.matmul(
                    ps[:],
                    lhsT=xv[:, k, t3 * 128:(t3 + 1) * 128],
                    rhs=wv[:, k, :],
                    start=(k == 0), stop=(k == 7))
            nc.vector.tensor_add(
                VP[:, t3, :].rearrange("p (h x) -> p h x", h=4)[:, :, 0:64],
                ps[:].rearrange("p (h x) -> p h x", h=4),
                bvb[:].rearrange("p (h x) -> p h x", h=4))

        # ---- Q projection (descending tt: attention qt=3 starts first) ----
        wq = load_w(wq_d, "wq")
        bqt = load_bias_cols(bq_d, "bqt")
        for tt in range(3, -1, -1):
            load_x_chunk(xq, xq_d, tt)
        for tt in range(3, -1, -1):
            for c in range(2):
                ps = psp.tile([128, 512], F32, tag="lg", bufs=3,
                              name=f"qps{c}_{tt}")
                for k in range(8):
                    nc.tensor.matmul(
                        ps[:],
                        lhsT=wq[:, k, c * 128:(c + 1) * 128],
                        rhs=xq[:, k, tt * 512:(tt + 1) * 512],
                        start=(k == 0), stop=(k == 7))
                nc.vector.tensor_scalar_add(
                    QT[:, c, tt * 512:(tt + 1) * 512],
                    ps[:], bqt[:, c:c + 1])

        mk = cpool.tile([128, 4, 512], F16, tag="mk")
        nc.sync.dma_start(mk[:], mask_d.ap().rearrange("k p m -> p k m"))
        wo = cpool.tile([128, 2, E], F16, tag="wo")
        nc.sync.dma_start(wo[:], wo_d.ap().rearrange("(c p) m -> p c m", p=128))

        # ---- O-projection emitters (interleaved into the next attention) --
        ots = {}

        def oproj_part(qt, tt, eo):
            if eo == 0:
                ots[tt] = opool.tile([128, E], F16, tag="ot", name=f"ot{tt}")
            ot = ots[tt]
            ps = psp.tile([128, 512], F32, tag="lg", bufs=3,
                          name=f"ops{tt}_{eo}")
            for c in range(2):
                nc.tensor.matmul(
                    ps[:],
                    lhsT=valsT[:, c, tt * 128:(tt + 1) * 128],
                    rhs=wo[:, c, eo * 512:(eo + 1) * 512],
                    start=(c == 0), stop=(c == 1))
            nc.vector.tensor_copy(ot[:, eo * 512:(eo + 1) * 512], ps[:])
            if eo == 1:
                nc.sync.dma_start(out_d.ap()[tt * 128:(tt + 1) * 128, :],
                                  ot[:])

        def oproj_emitters(qt):
            return [
                (lambda qt=qt, tt=tt, eo=eo: oproj_part(qt, tt, eo))
                for tt in range(4 * qt, 4 * qt + 4)
                for eo in range(2)
            ]

        # ---- attention per q-tile (qt descending), deferred O-proj ----
        def attention(qt, pending):
            nkt = 4 * qt + 4
            accs = {}
            for c in range(2):
                for hh in range(2):
                    accs[(c, hh)] = psp.tile([65, 512], F32, tag="acc",
                                             bufs=2, name=f"acc{qt}_{c}_{hh}")
            exs = {}

            def lg_exp(c, kt):
                lg = psp.tile([128, 2, 512], F32, tag="lg", bufs=3,
                              name=f"lg{qt}_{c}_{kt}")
                for hh in range(2):
                    nc.tensor.matmul(
                        lg[:, hh, :],
                        lhsT=KT[hh * 64:(hh + 1) * 64, c,
                                kt * 128:(kt + 1) * 128],
                        rhs=QT[hh * 64:(hh + 1) * 64, c,
                               qt * 512:(qt + 1) * 512],
                        start=True, stop=True)
                ex = expp.tile([128, 2, 512], F16, tag="ex",
                               name=f"ex{qt}_{c}_{kt}")
                nc.scalar.activation(ex[:, :, :], lg[:, :, :], AF.Exp,
                                     scale=0.125)
                dd = kt * 128 - qt * 512
                if dd >= 0:  # diagonal tile: multiplicative causal mask
                    for hh in range(2):
                        nc.vector.tensor_mul(ex[:, hh, :], ex[:, hh, :],
                                             mk[:, dd // 128, :])
                exs[(c, kt)] = ex

            def attn_v(c, kt):
                ex = exs.pop((c, kt))
                for hh in range(2):
                    h = 2 * c + hh
                    nc.tensor.matmul(
                        accs[(c, hh)][:],
                        lhsT=VP[:, kt, h * 66:h * 66 + 65],
                        rhs=ex[:, hh, :],
                        start=(kt == 0), stop=(kt == nkt - 1),
                        skip_group_check=True)

            steps = 2 * nkt
            per = max(1, -(-steps // len(pending))) if pending else steps + 1
            step = 0
            for c in range(2):
                for kt in range(nkt):
                    lg_exp(c, kt)
                    if kt >= 2:
                        attn_v(c, kt - 2)
                    step += 1
                    if pending and step % per == 0:
                        pending.pop(0)()
                attn_v(c, max(nkt - 2, 0))
                if nkt >= 2:
                    attn_v(c, nkt - 1)
            while pending:
                pending.pop(0)()
            return accs

        # normalize: reciprocal straight off the PSUM denominator row,
        # broadcast, fused PSUM-read multiply into valsT (releases acc)
        def normalize(qt, accs):
            for c in range(2):
                for hh in range(2):
                    acc = accs[(c, hh)]
                    rrow = smp.tile([1, 512], F32, tag="rrow",
                                    name=f"rrow{qt}_{c}_{hh}")
                    nc.vector.reciprocal(rrow[:], acc[64:65, :])
                    bc = smp.tile([64, 512], F32, tag="bc",
                                  name=f"bc{qt}_{c}_{hh}")
                    nc.gpsimd.partition_broadcast(bc[:], rrow[:])
                    nc.vector.tensor_mul(
                        valsT[hh * 64:(hh + 1) * 64, c,
                              qt * 512:(qt + 1) * 512],
                        acc[0:64, :], bc[:])

        prev_qt = None
        for qt in range(3, -1, -1):
            pending = oproj_emitters(prev_qt) if prev_qt is not None else []
            accs = attention(qt, pending)
            normalize(qt, accs)
            prev_qt = qt
        for fn in oproj_emitters(0):
            fn()

    nc.compile()
    return nc


def get_nc():
    if "nc" not in _CACHE:
        _CACHE["nc"] = _build()
    return _CACHE["nc"]


def _masks():
    i = np.arange(128)[:, None]
    j = np.arange(512)[None, :]
    m = np.zeros((4, 128, 512), dtype=NP16)
    for di in range(4):
        m[di] = (i + di * 128 <= j).astype(NP16)
    return m


def make_in_maps(query, key, value, Wq, bq, Wk, bk, Wv, bv, Wo, bo):
    query = np.asarray(query, np.float32)
    key = np.asarray(key, np.float32)
    value = np.asarray(value, np.float32)
    Wq, Wk, Wv, Wo = (np.asarray(a, np.float32) for a in (Wq, Wk, Wv, Wo))
    bq, bk, bv = (np.asarray(a, np.float32) for a in (bq, bk, bv))
    masks = _masks()
    in_maps = []
    for c in range(NCORES):
        b, g = divmod(c, 4)
        sl = slice(g * EL, (g + 1) * EL)
        in_maps.append({
            "xqT": np.ascontiguousarray(query[b].T).astype(NP16),
            "xkT": np.ascontiguousarray(key[b].T).astype(NP16),
            "xvT": np.ascontiguousarray(value[b].T).astype(NP16),
            "wqT": np.ascontiguousarray(Wq[sl, :].T).astype(NP16),
            "wkT": np.ascontiguousarray(Wk[sl, :].T).astype(NP16),
            "wvT": np.ascontiguousarray(Wv[sl, :].T).astype(NP16),
            "woT": np.ascontiguousarray(Wo[:, sl].T).astype(NP16),
            "bq": np.ascontiguousarray(bq[sl]),
            "bk": np.ascontiguousarray(bk[sl]),
            "bv": np.ascontiguousarray(bv[sl]),
            "masks": masks,
        })
    return in_maps


def run(inputs, trace=False, tmpdir=None):
    """Run on 8 cores; returns (full_output, BassKernelResults)."""
    nc = get_nc()
    in_maps = make_in_maps(**inputs)
    res = bass_utils.run_bass_kernel_spmd(
        nc, in_maps, list(range(NCORES)), trace=trace, tmpdir=tmpdir)
    bo = np.asarray(inputs["bo"], np.float32)
    out = np.zeros((B, S, E), np.float32)
    for c in range(NCORES):
        out[c // 4] += res.results[c]["out"].astype(np.float32)
    out += bo[None, None, :]
    return out, res


def kernel(**inputs):
    out, _ = run(inputs)
    return out


# revision 6
# speedup vs baseline: 1.2477x; 1.2477x over previous
"""Multi-head causal attention (B=2, S=2048, E=1024, H=16, D=64) on 8 TRN2
NeuronCores.

Sharding (data + tensor parallel, Megatron-style):
  core c -> batch b = c // 4, head group g = c % 4 (4 heads, e' = 256 cols).
  Wq/Wk/Wv column-sharded ([256, 1024] slices), Wo row-sharded
  ([1024, 256] slice); each core produces a partial output [2048, 1024]
  (fp16) which the host sums per batch group in fp32 (the Megatron
  all-reduce) and adds bo.

Per-core device kernel (matmul operands fp16, accumulate fp32 in PSUM):
  Inputs stream in 512-column chunks, host-prepacked [tt, p, k, m] so each
  chunk is one 8KB-contiguous-per-partition DMA, ordered by consumption
  (K, V, Q desc) so projections start ~4us in and hide the 12MB input DMA.
  K^T = Wk_l x_k^T + bk  [256, 2048], Q^T likewise (tt descending so
  attention qt=3 starts first), V' = [x_v Wv_l^T + bv | 1] with the ones
  column memset (softmax denominator via PE).
  Attention in S^T orientation per (q-tile 512, head-pair chunk): S^T
  tiles [128 k, 512 q] with 2 heads in PE row groups 0-63/64-127, exp on
  ACT (1/8 scale folded), multiplicative causal mask on diagonal tiles.
  Diagonal tiles only touch live columns [dd:512] in the logits matmul,
  exp, mask, and attnV (the dead columns are never written or read).
  acc += V'^T @ P^T in PSUM [65, 512] whose row 64 is the denominator.
  Normalize without DMA: DVE reciprocal_approx_fast on the PSUM
  denominator row, GpSimd partition-broadcast, fused PSUM-read multiply
  into valsT (fp16). Each qt's O-projection is deferred and interleaved
  into the NEXT qt's attention k-loop (the inner loop is ACT-bound, so
  the PE absorbs the O-proj matmuls); PSUM->SBUF O copies run on GpSimd;
  fp16 outputs DMA per 128-row tile.
"""
import sys
import os

sys.path.insert(0, "/opt/trn_rl_repo")

import numpy as np
from contextlib import ExitStack

import concourse.bass as bass  # noqa: E402
import concourse.mybir as mybir  # noqa: E402
import concourse.tile as tile  # noqa: E402
from concourse import bacc, bass_utils  # noqa: E402

bass_utils.upload_artifacts = lambda d: f"local:{d}"

B, S, E, H, D = 2, 2048, 1024, 16, 64
NCORES = 8
EL = 256  # e' columns per core (4 heads)
F32 = mybir.dt.float32
F16 = mybir.dt.float16
AF = mybir.ActivationFunctionType
NP16 = np.float16

_CACHE = {}


def _build():
    nc = bacc.Bacc("TRN2", target_bir_lowering=False, debug=False)

    xq_d = nc.dram_tensor("xq4", [4, 128, 8, 512], F16, kind="ExternalInput")
    xk_d = nc.dram_tensor("xk4", [4, 128, 8, 512], F16, kind="ExternalInput")
    xv_d = nc.dram_tensor("xv4", [4, 128, 8, 512], F16, kind="ExternalInput")
    wq_d = nc.dram_tensor("wqT", [E, EL], F16, kind="ExternalInput")
    wk_d = nc.dram_tensor("wkT", [E, EL], F16, kind="ExternalInput")
    wv_d = nc.dram_tensor("wvT", [E, EL], F16, kind="ExternalInput")
    wo_d = nc.dram_tensor("woT", [EL, E], F16, kind="ExternalInput")
    bq_d = nc.dram_tensor("bq", [EL], F32, kind="ExternalInput")
    bk_d = nc.dram_tensor("bk", [EL], F32, kind="ExternalInput")
    bv_d = nc.dram_tensor("bv", [EL], F32, kind="ExternalInput")
    mask_d = nc.dram_tensor("masks", [4, 128, 512], F16, kind="ExternalInput")
    out_d = nc.dram_tensor("out", [S, E], F16, kind="ExternalOutput")

    with tile.TileContext(nc) as tc, ExitStack() as ctx:
        cpool = ctx.enter_context(tc.tile_pool(name="const", bufs=1))
        psp = ctx.enter_context(tc.tile_pool(name="psp", bufs=2, space="PSUM"))
        expp = ctx.enter_context(tc.tile_pool(name="expp", bufs=6))
        opool = ctx.enter_context(tc.tile_pool(name="op", bufs=4))
        smp = ctx.enter_context(tc.tile_pool(name="smp", bufs=4))

        xk = cpool.tile([128, 4, 8, 512], F16, tag="xk")
        xq = cpool.tile([128, 4, 8, 512], F16, tag="xq")
        xv = cpool.tile([128, 4, 8, 512], F16, tag="xv")
        KT = cpool.tile([128, 2, S], F16, tag="KT")
        QT = cpool.tile([128, 2, S], F16, tag="QT")
        VP = cpool.tile([128, 16, 4 * 66], F16, tag="VP")  # 66: 4B-aligned
        valsT = cpool.tile([128, 2, S], F16, tag="valsT")

        def load_w(dram, tag):
            w = cpool.tile([128, 8, EL], F16, tag=tag)
            nc.sync.dma_start(w[:], dram.ap().rearrange("(k p) m -> p k m", p=128))
            return w

        def load_bias_cols(dram, tag):
            b = cpool.tile([128, 2], F32, tag=tag)
            nc.sync.dma_start(b[:], dram.ap().rearrange("(c p) -> p c", p=128))
            return b

        def load_x_chunk(x_t, dram, tt):
            nc.sync.dma_start(
                x_t[:, tt, :, :],
                dram.ap()[tt:tt + 1].rearrange("t p k m -> p (t k) m"),
            )

        # ---- K projection (streamed): K^T[e', t] = Wk x_k^T + bk ----
        wk = load_w(wk_d, "wk")
        bkt = load_bias_cols(bk_d, "bkt")
        for tt in range(4):
            load_x_chunk(xk, xk_d, tt)
        for tt in range(4):
            for c in range(2):
                ps = psp.tile([128, 512], F32, tag="lg", bufs=3,
                              name=f"kps{c}_{tt}")
                for k in range(8):
                    nc.tensor.matmul(
                        ps[:],
                        lhsT=wk[:, k, c * 128:(c + 1) * 128],
                        rhs=xk[:, tt, k, :],
                        start=(k == 0), stop=(k == 7))
                nc.vector.tensor_scalar_add(
                    KT[:, c, tt * 512:(tt + 1) * 512],
                    ps[:], bkt[:, c:c + 1])

        # ---- V projection (natural layout) ----
        wv = load_w(wv_d, "wv")
        bvr = cpool.tile([1, EL], F32, tag="bvr")
        nc.sync.dma_start(bvr[:], bv_d.ap().rearrange("(p m) -> p m", p=1))
        bvb = cpool.tile([128, EL], F32, tag="bvb")
        nc.gpsimd.partition_broadcast(bvb[:], bvr[:])
        # ones columns of V' (col 64 of each 66-block)
        nc.vector.memset(
            VP[:].rearrange("p k (h x) -> p k h x", h=4)[:, :, :, 64:65], 1.0)
        for tt in range(4):
            load_x_chunk(xv, xv_d, tt)
        for t3 in range(16):
            tt, t3r = divmod(t3, 4)
            ps = psp.tile([128, EL], F32, tag="lg", bufs=3, name=f"vps{t3}")
            for k in range(8):
                nc.tensor.matmul(
                    ps[:],
                    lhsT=xv[:, tt, k, t3r * 128:(t3r + 1) * 128],
                    rhs=wv[:, k, :],
                    start=(k == 0), stop=(k == 7))
            nc.vector.tensor_add(
                VP[:, t3, :].rearrange("p (h x) -> p h x", h=4)[:, :, 0:64],
                ps[:].rearrange("p (h x) -> p h x", h=4),
                bvb[:].rearrange("p (h x) -> p h x", h=4))

        # ---- Q projection (descending tt: attention qt=3 starts first) ----
        wq = load_w(wq_d, "wq")
        bqt = load_bias_cols(bq_d, "bqt")
        for tt in range(3, -1, -1):
            load_x_chunk(xq, xq_d, tt)
        for tt in range(3, -1, -1):
            for c in range(2):
                ps = psp.tile([128, 512], F32, tag="lg", bufs=3,
                              name=f"qps{c}_{tt}")
                for k in range(8):
                    nc.tensor.matmul(
                        ps[:],
                        lhsT=wq[:, k, c * 128:(c + 1) * 128],
                        rhs=xq[:, tt, k, :],
                        start=(k == 0), stop=(k == 7))
                nc.vector.tensor_scalar_add(
                    QT[:, c, tt * 512:(tt + 1) * 512],
                    ps[:], bqt[:, c:c + 1])

        mk = cpool.tile([128, 4, 512], F16, tag="mk")
        nc.sync.dma_start(mk[:], mask_d.ap().rearrange("k p m -> p k m"))
        wo = cpool.tile([128, 2, E], F16, tag="wo")
        nc.sync.dma_start(wo[:], wo_d.ap().rearrange("(c p) m -> p c m", p=128))

        # ---- O-projection emitters (interleaved into the next attention) --
        ots = {}

        def oproj_part(qt, tt, eo):
            if eo == 0:
                ots[tt] = opool.tile([128, E], F16, tag="ot", name=f"ot{tt}")
            ot = ots[tt]
            ps = psp.tile([128, 512], F32, tag="lg", bufs=3,
                          name=f"ops{tt}_{eo}")
            for c in range(2):
                nc.tensor.matmul(
                    ps[:],
                    lhsT=valsT[:, c, tt * 128:(tt + 1) * 128],
                    rhs=wo[:, c, eo * 512:(eo + 1) * 512],
                    start=(c == 0), stop=(c == 1))
            nc.vector.tensor_copy(ot[:, eo * 512:(eo + 1) * 512], ps[:])
            if eo == 1:
                nc.sync.dma_start(out_d.ap()[tt * 128:(tt + 1) * 128, :],
                                  ot[:])

        def oproj_emitters(qt):
            return [
                (lambda qt=qt, tt=tt, eo=eo: oproj_part(qt, tt, eo))
                for tt in range(4 * qt, 4 * qt + 4)
                for eo in range(2)
            ]

        # ---- attention per q-tile (qt descending), deferred O-proj ----
        def attention(qt, pending):
            nkt = 4 * qt + 4
            accs = {}
            for c in range(2):
                for hh in range(2):
                    accs[(c, hh)] = psp.tile([65, 512], F32, tag="acc",
                                             bufs=2, name=f"acc{qt}_{c}_{hh}")
            exs = {}

            def lg_exp(c, kt):
                dd = max(kt * 128 - qt * 512, 0)  # first live column
                lg = psp.tile([128, 2, 512], F32, tag="lg", bufs=3,
                              name=f"lg{qt}_{c}_{kt}")
                for hh in range(2):
                    nc.tensor.matmul(
                        lg[:, hh, dd:],
                        lhsT=KT[hh * 64:(hh + 1) * 64, c,
                                kt * 128:(kt + 1) * 128],
                        rhs=QT[hh * 64:(hh + 1) * 64, c,
                               qt * 512 + dd:(qt + 1) * 512],
                        start=True, stop=True)
                ex = expp.tile([128, 2, 512], F16, tag="ex",
                               name=f"ex{qt}_{c}_{kt}")
                nc.scalar.activation(ex[:, :, dd:], lg[:, :, dd:], AF.Exp,
                                     scale=0.125)
                if kt * 128 - qt * 512 >= 0:  # diagonal: triangular mask
                    for hh in range(2):
                        nc.vector.tensor_mul(ex[:, hh, dd:], ex[:, hh, dd:],
                                             mk[:, dd // 128, dd:])
                exs[(c, kt)] = (ex, dd)

            def attn_v(c, kt):
                ex, dd = exs.pop((c, kt))
                for hh in range(2):
                    h = 2 * c + hh
                    nc.tensor.matmul(
                        accs[(c, hh)][:, dd:],
                        lhsT=VP[:, kt, h * 66:h * 66 + 65],
                        rhs=ex[:, hh, dd:],
                        start=(kt == 0), stop=(kt == nkt - 1),
                        skip_group_check=True)

            steps = 2 * nkt
            per = max(1, -(-steps // len(pending))) if pending else steps + 1
            step = 0
            for c in range(2):
                for kt in range(nkt):
                    lg_exp(c, kt)
                    if kt >= 2:
                        attn_v(c, kt - 2)
                    step += 1
                    if pending and step % per == 0:
                        pending.pop(0)()
                attn_v(c, max(nkt - 2, 0))
                if nkt >= 2:
                    attn_v(c, nkt - 1)
            while pending:
                pending.pop(0)()
            return accs

        # normalize: reciprocal_approx_fast straight off the PSUM denominator
        # row, broadcast, fused PSUM-read multiply into valsT (releases acc)
        def normalize(qt, accs):
            for c in range(2):
                for hh in range(2):
                    acc = accs[(c, hh)]
                    drow = smp.tile([1, 512], F32, tag="drow",
                                    name=f"drow{qt}_{c}_{hh}")
                    nc.vector.tensor_copy(drow[:], acc[64:65, :])
                    rrow = smp.tile([1, 512], F32, tag="rrow",
                                    name=f"rrow{qt}_{c}_{hh}")
                    nc.vector.reciprocal_approx_fast(rrow[:], drow[:])
                    bc = smp.tile([64, 512], F32, tag="bc",
                                  name=f"bc{qt}_{c}_{hh}")
                    nc.gpsimd.partition_broadcast(bc[:], rrow[:])
                    nc.vector.tensor_mul(
                        valsT[hh * 64:(hh + 1) * 64, c,
                              qt * 512:(qt + 1) * 512],
                        acc[0:64, :], bc[:])

        prev_qt = None
        for qt in range(3, -1, -1):
            pending = oproj_emitters(prev_qt) if prev_qt is not None else []
            accs = attention(qt, pending)
            normalize(qt, accs)
            prev_qt = qt
        for fn in oproj_emitters(0):
            fn()

    nc.compile()
    return nc


def get_nc():
    if "nc" not in _CACHE:
        _CACHE["nc"] = _build()
    return _CACHE["nc"]


def _masks():
    i = np.arange(128)[:, None]
    j = np.arange(512)[None, :]
    m = np.zeros((4, 128, 512), dtype=NP16)
    for di in range(4):
        m[di] = (i + di * 128 <= j).astype(NP16)
    return m


def _pack_x(xt):
    # [E, S] f32 -> [4(tt), 128(p), 8(k), 512(m)] f16, 8KB contiguous chunks
    return np.ascontiguousarray(
        xt.reshape(8, 128, 4, 512).transpose(2, 1, 0, 3)).astype(NP16)


def make_in_maps(query, key, value, Wq, bq, Wk, bk, Wv, bv, Wo, bo):
    query = np.asarray(query, np.float32)
    key = np.asarray(key, np.float32)
    value = np.asarray(value, np.float32)
    Wq, Wk, Wv, Wo = (np.asarray(a, np.float32) for a in (Wq, Wk, Wv, Wo))
    bq, bk, bv = (np.asarray(a, np.float32) for a in (bq, bk, bv))
    masks = _masks()
    in_maps = []
    for c in range(NCORES):
        b, g = divmod(c, 4)
        sl = slice(g * EL, (g + 1) * EL)
        in_maps.append({
            "xq4": _pack_x(query[b].T),
            "xk4": _pack_x(key[b].T),
            "xv4": _pack_x(value[b].T),
            "wqT": np.ascontiguousarray(Wq[sl, :].T).astype(NP16),
            "wkT": np.ascontiguousarray(Wk[sl, :].T).astype(NP16),
            "wvT": np.ascontiguousarray(Wv[sl, :].T).astype(NP16),
            "woT": np.ascontiguousarray(Wo[:, sl].T).astype(NP16),
            "bq": np.ascontiguousarray(bq[sl]),
            "bk": np.ascontiguousarray(bk[sl]),
            "bv": np.ascontiguousarray(bv[sl]),
            "masks": masks,
        })
    return in_maps


def run(inputs, trace=False, tmpdir=None):
    """Run on 8 cores; returns (full_output, BassKernelResults)."""
    nc = get_nc()
    in_maps = make_in_maps(**inputs)
    res = bass_utils.run_bass_kernel_spmd(
        nc, in_maps, list(range(NCORES)), trace=trace, tmpdir=tmpdir)
    bo = np.asarray(inputs["bo"], np.float32)
    out = np.zeros((B, S, E), np.float32)
    for c in range(NCORES):
        out[c // 4] += res.results[c]["out"].astype(np.float32)
    out += bo[None, None, :]
    return out, res


def kernel(**inputs):
    out, _ = run(inputs)
    return out


# revision 12
# speedup vs baseline: 1.3028x; 1.0442x over previous
"""Multi-head causal attention (B=2, S=2048, E=1024, H=16, D=64) on 8 TRN2
NeuronCores.

Sharding (data + tensor parallel, Megatron-style):
  core c -> batch b = c // 4, head group g = c % 4 (4 heads, e' = 256 cols).
  Wq/Wk/Wv column-sharded ([256, 1024] slices), Wo row-sharded
  ([1024, 256] slice); each core produces a partial output [2048, 1024]
  (fp16) which the host sums per batch group in fp32 (the Megatron
  all-reduce) and adds bo.

Per-core device kernel (matmul operands fp16, accumulate fp32 in PSUM):
  Inputs stream in 512-column chunks, host-prepacked [tt, p, k, m] so each
  chunk is one 8KB-contiguous-per-partition DMA, ordered by consumption
  (K, V, Q desc) so projections start ~4us in and hide the 12MB input DMA.
  K^T = Wk_l x_k^T + bk  [256, 2048], Q^T likewise (tt descending so
  attention qt=3 starts first), V' = [x_v Wv_l^T + bv | 1] with the ones
  column memset (softmax denominator via PE).
  Attention in S^T orientation per (q-tile 512, head-pair chunk): S^T
  tiles [128 k, 512 q] with 2 heads in PE row groups 0-63/64-127, exp on
  ACT (1/8 scale folded), multiplicative causal mask on diagonal tiles.
  Diagonal tiles only touch live columns [dd:512] in the logits matmul,
  exp, mask, and attnV (the dead columns are never written or read).
  acc += V'^T @ P^T in PSUM [65, 512] whose row 64 is the denominator.
  Normalize without DMA: DVE reciprocal_approx_fast on the PSUM
  denominator row, GpSimd partition-broadcast, fused PSUM-read multiply
  into valsT (fp16). Each qt's O-projection is deferred and interleaved
  into the NEXT qt's attention k-loop (the inner loop is ACT-bound, so
  the PE absorbs the O-proj matmuls); PSUM->SBUF O copies run on GpSimd;
  fp16 outputs DMA per 128-row tile.
"""
import sys
import os

sys.path.insert(0, "/opt/trn_rl_repo")

import numpy as np
from contextlib import ExitStack

import concourse.bass as bass  # noqa: E402
import concourse.mybir as mybir  # noqa: E402
import concourse.tile as tile  # noqa: E402
from concourse import bacc, bass_utils  # noqa: E402

bass_utils.upload_artifacts = lambda d: f"local:{d}"

B, S, E, H, D = 2, 2048, 1024, 16, 64
NCORES = 8
EL = 256  # e' columns per core (4 heads)
F32 = mybir.dt.float32
F16 = mybir.dt.float16
AF = mybir.ActivationFunctionType
NP16 = np.float16

_CACHE = {}


def _build():
    nc = bacc.Bacc("TRN2", target_bir_lowering=False, debug=False)

    xq_d = nc.dram_tensor("xq4", [4, 128, 8, 512], F16, kind="ExternalInput")
    xk_d = nc.dram_tensor("xk4", [4, 128, 8, 512], F16, kind="ExternalInput")
    xv_d = nc.dram_tensor("xv4", [4, 128, 8, 512], F16, kind="ExternalInput")
    wq_d = nc.dram_tensor("wqT", [E, EL], F16, kind="ExternalInput")
    wk_d = nc.dram_tensor("wkT", [E, EL], F16, kind="ExternalInput")
    wv_d = nc.dram_tensor("wvT", [E, EL], F16, kind="ExternalInput")
    wo_d = nc.dram_tensor("woT", [EL, E], F16, kind="ExternalInput")
    bq_d = nc.dram_tensor("bq", [EL], F32, kind="ExternalInput")
    bk_d = nc.dram_tensor("bk", [EL], F32, kind="ExternalInput")
    bv_d = nc.dram_tensor("bv", [EL], F32, kind="ExternalInput")
    mask_d = nc.dram_tensor("masks", [4, 128, 512], F16, kind="ExternalInput")
    out_d = nc.dram_tensor("out", [S, E], F16, kind="ExternalOutput")

    with tile.TileContext(nc) as tc, ExitStack() as ctx:
        cpool = ctx.enter_context(tc.tile_pool(name="const", bufs=1))
        psp = ctx.enter_context(tc.tile_pool(name="psp", bufs=2, space="PSUM"))
        expp = ctx.enter_context(tc.tile_pool(name="expp", bufs=8))
        opool = ctx.enter_context(tc.tile_pool(name="op", bufs=4))
        smp = ctx.enter_context(tc.tile_pool(name="smp", bufs=4))

        xk = cpool.tile([128, 4, 8, 512], F16, tag="xk")
        xq = cpool.tile([128, 4, 8, 512], F16, tag="xq")
        xv = cpool.tile([128, 4, 8, 512], F16, tag="xv")
        KT = cpool.tile([128, 2, S], F16, tag="KT")
        QT = cpool.tile([128, 2, S], F16, tag="QT")
        VP = cpool.tile([128, 16, 4 * 66], F16, tag="VP")  # 66: 4B-aligned
        valsT = cpool.tile([128, 2, S], F16, tag="valsT")

        def load_w(dram, tag):
            w = cpool.tile([128, 8, EL], F16, tag=tag)
            nc.sync.dma_start(w[:], dram.ap().rearrange("(k p) m -> p k m", p=128))
            return w

        def load_bias_cols(dram, tag):
            b = cpool.tile([128, 2], F32, tag=tag)
            nc.sync.dma_start(b[:], dram.ap().rearrange("(c p) -> p c", p=128))
            return b

        def load_x_chunk(x_t, dram, tt):
            nc.sync.dma_start(
                x_t[:, tt, :, :],
                dram.ap()[tt:tt + 1].rearrange("t p k m -> p (t k) m"),
            )

        # ---- K projection (streamed): K^T[e', t] = Wk x_k^T + bk ----
        wk = load_w(wk_d, "wk")
        bkt = load_bias_cols(bk_d, "bkt")
        for tt in range(4):
            load_x_chunk(xk, xk_d, tt)
        for tt in range(4):
            for c in range(2):
                ps = psp.tile([128, 512], F32, tag="lg", bufs=3,
                              name=f"kps{c}_{tt}")
                for k in range(8):
                    nc.tensor.matmul(
                        ps[:],
                        lhsT=wk[:, k, c * 128:(c + 1) * 128],
                        rhs=xk[:, tt, k, :],
                        start=(k == 0), stop=(k == 7))
                nc.vector.tensor_scalar_add(
                    KT[:, c, tt * 512:(tt + 1) * 512],
                    ps[:], bkt[:, c:c + 1])

        # ---- V projection (natural layout) ----
        wv = load_w(wv_d, "wv")
        bvr = cpool.tile([1, EL], F32, tag="bvr")
        nc.sync.dma_start(bvr[:], bv_d.ap().rearrange("(p m) -> p m", p=1))
        bvb = cpool.tile([128, EL], F32, tag="bvb")
        nc.gpsimd.partition_broadcast(bvb[:], bvr[:])
        # ones columns of V' (col 64 of each 66-block)
        nc.vector.memset(
            VP[:].rearrange("p k (h x) -> p k h x", h=4)[:, :, :, 64:65], 1.0)
        for tt in range(4):
            load_x_chunk(xv, xv_d, tt)
        for t3 in range(16):
            tt, t3r = divmod(t3, 4)
            ps = psp.tile([128, EL], F32, tag="lg", bufs=3, name=f"vps{t3}")
            for k in range(8):
                nc.tensor.matmul(
                    ps[:],
                    lhsT=xv[:, tt, k, t3r * 128:(t3r + 1) * 128],
                    rhs=wv[:, k, :],
                    start=(k == 0), stop=(k == 7))
            nc.vector.tensor_add(
                VP[:, t3, :].rearrange("p (h x) -> p h x", h=4)[:, :, 0:64],
                ps[:].rearrange("p (h x) -> p h x", h=4),
                bvb[:].rearrange("p (h x) -> p h x", h=4))

        # ---- Q projection (descending tt: attention qt=3 starts first; only
        # tt=3 is emitted here — the rest interleave into attention(3)) ----
        wq = load_w(wq_d, "wq")
        bqt = load_bias_cols(bq_d, "bqt")
        for tt in range(3, -1, -1):
            load_x_chunk(xq, xq_d, tt)

        def qproj_part(tt, c):
            ps = psp.tile([128, 512], F32, tag="lg", bufs=3,
                          name=f"qps{c}_{tt}")
            for k in range(8):
                nc.tensor.matmul(
                    ps[:],
                    lhsT=wq[:, k, c * 128:(c + 1) * 128],
                    rhs=xq[:, tt, k, :],
                    start=(k == 0), stop=(k == 7))
            nc.vector.tensor_scalar_add(
                QT[:, c, tt * 512:(tt + 1) * 512],
                ps[:], bqt[:, c:c + 1])

        for c in range(2):
            qproj_part(3, c)
        qproj_pending = [
            (lambda tt=tt, c=c: qproj_part(tt, c))
            for tt in (2, 1, 0) for c in range(2)
        ]

        mk = cpool.tile([128, 4, 512], F16, tag="mk")
        nc.sync.dma_start(mk[:], mask_d.ap().rearrange("k p m -> p k m"))
        wo = cpool.tile([128, 2, E], F16, tag="wo")
        nc.sync.dma_start(wo[:], wo_d.ap().rearrange("(c p) m -> p c m", p=128))

        # ---- O-projection emitters (interleaved into the next attention) --
        ots = {}

        def oproj_part(qt, tt, eo):
            if eo == 0:
                ots[tt] = opool.tile([128, E], F16, tag="ot", name=f"ot{tt}")
            ot = ots[tt]
            ps = psp.tile([128, 512], F32, tag="lg", bufs=3,
                          name=f"ops{tt}_{eo}")
            for c in range(2):
                nc.tensor.matmul(
                    ps[:],
                    lhsT=valsT[:, c, tt * 128:(tt + 1) * 128],
                    rhs=wo[:, c, eo * 512:(eo + 1) * 512],
                    start=(c == 0), stop=(c == 1))
            nc.vector.tensor_copy(ot[:, eo * 512:(eo + 1) * 512], ps[:])
            if eo == 1:
                nc.sync.dma_start(out_d.ap()[tt * 128:(tt + 1) * 128, :],
                                  ot[:])

        def oproj_emitters(qt):
            return [
                (lambda qt=qt, tt=tt, eo=eo: oproj_part(qt, tt, eo))
                for tt in range(4 * qt, 4 * qt + 4)
                for eo in range(2)
            ]

        # ---- attention per q-tile (qt descending), deferred O-proj ----
        def attention(qt, pending):
            nkt = 4 * qt + 4
            accs = {}
            for c in range(2):
                for hh in range(2):
                    accs[(c, hh)] = psp.tile([65, 512], F32, tag="acc",
                                             bufs=2, name=f"acc{qt}_{c}_{hh}")
            exs = {}

            def lg_exp(c, kt):
                dd = max(kt * 128 - qt * 512, 0)  # first live column
                lg = psp.tile([128, 2, 512], F32, tag="lg", bufs=3,
                              name=f"lg{qt}_{c}_{kt}")
                for hh in range(2):
                    nc.tensor.matmul(
                        lg[:, hh, dd:],
                        lhsT=KT[hh * 64:(hh + 1) * 64, c,
                                kt * 128:(kt + 1) * 128],
                        rhs=QT[hh * 64:(hh + 1) * 64, c,
                               qt * 512 + dd:(qt + 1) * 512],
                        start=True, stop=True)
                ex = expp.tile([128, 2, 512], F16, tag="ex",
                               name=f"ex{qt}_{c}_{kt}")
                nc.scalar.activation(ex[:, :, dd:], lg[:, :, dd:], AF.Exp,
                                     scale=0.125)
                if kt * 128 - qt * 512 >= 0:  # diagonal: triangular mask
                    # beyond col dd+128 the mask is all-ones — multiply only
                    # the 128 columns containing the triangle
                    for hh in range(2):
                        nc.vector.tensor_mul(
                            ex[:, hh, dd:dd + 128], ex[:, hh, dd:dd + 128],
                            mk[:, dd // 128, dd:dd + 128])
                exs[(c, kt)] = (ex, dd)

            def attn_v(c, kt):
                ex, dd = exs.pop((c, kt))
                for hh in range(2):
                    h = 2 * c + hh
                    nc.tensor.matmul(
                        accs[(c, hh)][:, dd:],
                        lhsT=VP[:, kt, h * 66:h * 66 + 65],
                        rhs=ex[:, hh, dd:],
                        start=(kt == 0), stop=(kt == nkt - 1),
                        skip_group_check=True)

            steps = 2 * nkt
            per = max(1, -(-steps // len(pending))) if pending else steps + 1
            step = 0
            for c in range(2):
                for kt in range(nkt):
                    lg_exp(c, kt)
                    if kt >= 2:
                        attn_v(c, kt - 2)
                    step += 1
                    if pending and step % per == 0:
                        pending.pop(0)()
                attn_v(c, max(nkt - 2, 0))
                if nkt >= 2:
                    attn_v(c, nkt - 1)
            while pending:
                pending.pop(0)()
            return accs

        # normalize, stage-major for cross-engine pipelining: denominator-row
        # copies first (release acc slots early), then approx reciprocals,
        # broadcasts (GpSimd), and fused PSUM-read multiplies into valsT
        def normalize(qt, accs):
            chans = [(c, hh) for c in range(2) for hh in range(2)]
            drows, rrows, bcs = {}, {}, {}
            for c, hh in chans:
                drows[(c, hh)] = smp.tile([1, 512], F32, tag="drow",
                                          name=f"drow{qt}_{c}_{hh}")
                nc.vector.tensor_copy(drows[(c, hh)][:],
                                      accs[(c, hh)][64:65, :])
                rrows[(c, hh)] = smp.tile([1, 512], F32, tag="rrow",
                                          name=f"rrow{qt}_{c}_{hh}")
                nc.vector.reciprocal_approx_fast(rrows[(c, hh)][:],
                                                 drows[(c, hh)][:])
                bcs[(c, hh)] = smp.tile([64, 512], F32, tag="bc",
                                        name=f"bc{qt}_{c}_{hh}")
                nc.gpsimd.partition_broadcast(bcs[(c, hh)][:],
                                              rrows[(c, hh)][:])
            for c, hh in chans:
                nc.vector.tensor_mul(
                    valsT[hh * 64:(hh + 1) * 64, c,
                          qt * 512:(qt + 1) * 512],
                    accs[(c, hh)][0:64, :], bcs[(c, hh)][:])

        prev_qt = None
        for qt in range(3, -1, -1):
            pending = oproj_emitters(prev_qt) if prev_qt is not None else []
            if qt == 3:
                pending = qproj_pending
            accs = attention(qt, pending)
            normalize(qt, accs)
            prev_qt = qt
        for fn in oproj_emitters(0):
            fn()

    nc.compile()
    return nc


def get_nc():
    if "nc" not in _CACHE:
        _CACHE["nc"] = _build()
    return _CACHE["nc"]


def _masks():
    i = np.arange(128)[:, None]
    j = np.arange(512)[None, :]
    m = np.zeros((4, 128, 512), dtype=NP16)
    for di in range(4):
        m[di] = (i + di * 128 <= j).astype(NP16)
    return m


def _pack_x(xt):
    # [E, S] f32 -> [4(tt), 128(p), 8(k), 512(m)] f16, 8KB contiguous chunks
    return np.ascontiguousarray(
        xt.reshape(8, 128, 4, 512).transpose(2, 1, 0, 3)).astype(NP16)


def make_in_maps(query, key, value, Wq, bq, Wk, bk, Wv, bv, Wo, bo):
    query = np.asarray(query, np.float32)
    key = np.asarray(key, np.float32)
    value = np.asarray(value, np.float32)
    Wq, Wk, Wv, Wo = (np.asarray(a, np.float32) for a in (Wq, Wk, Wv, Wo))
    bq, bk, bv = (np.asarray(a, np.float32) for a in (bq, bk, bv))
    masks = _masks()
    in_maps = []
    for c in range(NCORES):
        b, g = divmod(c, 4)
        sl = slice(g * EL, (g + 1) * EL)
        in_maps.append({
            "xq4": _pack_x(query[b].T),
            "xk4": _pack_x(key[b].T),
            "xv4": _pack_x(value[b].T),
            "wqT": np.ascontiguousarray(Wq[sl, :].T).astype(NP16),
            "wkT": np.ascontiguousarray(Wk[sl, :].T).astype(NP16),
            "wvT": np.ascontiguousarray(Wv[sl, :].T).astype(NP16),
            "woT": np.ascontiguousarray(Wo[:, sl].T).astype(NP16),
            "bq": np.ascontiguousarray(bq[sl]),
            "bk": np.ascontiguousarray(bk[sl]),
            "bv": np.ascontiguousarray(bv[sl]),
            "masks": masks,
        })
    return in_maps


def run(inputs, trace=False, tmpdir=None):
    """Run on 8 cores; returns (full_output, BassKernelResults)."""
    nc = get_nc()
    in_maps = make_in_maps(**inputs)
    res = bass_utils.run_bass_kernel_spmd(
        nc, in_maps, list(range(NCORES)), trace=trace, tmpdir=tmpdir)
    bo = np.asarray(inputs["bo"], np.float32)
    out = np.zeros((B, S, E), np.float32)
    for c in range(NCORES):
        out[c // 4] += res.results[c]["out"].astype(np.float32)
    out += bo[None, None, :]
    return out, res


def kernel(**inputs):
    out, _ = run(inputs)
    return out
